# revision 1
# baseline (speedup 1.0000x reference)
"""Trainium2 Bass kernel for AdditiveLowRankPairwise.

scores[b,t,s] = sum_r iw[r]*silu(pt[b,t,r]*ps[b,s,r]) + tl[b,t] + sl[b,s] + bias
  pt = target_val @ Wt.T   [B,T,R]
  ps = source_val @ Ws.T   [B,S,R]
  tl = pt @ wt_out         [B,T]
  sl = ps @ ws_out         [B,S]

B=2, T=S=1024, D=512, R=64.  8 cores: core c handles b=c//4, t-rows
[(c%4)*256, (c%4+1)*256).  Per core:
  - ps2 [128,1024] (r-duplicated on partition halves) is produced directly in
    PSUM by the projection matmuls using a host-duplicated stationary
    wsT2=[WsT|WsT]; it stays resident in PSUM for the whole kernel.
  - per pair p of t-rows (t_top=tb*128+p, t_bot=tb*128+64+p):
      one ACT instruction computes silu(ps2[q,s]*pt2[q,p]) via the
      per-partition scale operand (PSUM source), f32 output;
      PE matmul (float32r: full-rate, ~fp32 precision) with a
      2-one-hot-column stationary (slice of a host-built [128,192] matrix)
      accumulates the iw-weighted partition-reduction into score psum rows
      {p, 64+p}.
  - sl broadcast folded in via psum-initializing matmul (ws_out replicated
    stationary); tl+bias added in the PSUM->SBUF fixup (per-partition bias).

loop_n>0 wraps the body in an on-device For_i loop (wall-clock-delta timing).
"""

import numpy as np

B, T, S, D, R = 2, 1024, 1024, 512, 64
TBLK = 256          # t-rows per core
NCORES = 8
GRP = 8             # pairs per activation batch (dve_prod variant)
VARIANT = "act_fused"
_ACT_NAME = "Silu"  # sim override: CoreSim lacks Silu; tests may set "Sigmoid"

_compiled = {}


def _build_nc(variant=VARIANT, loop_n=0):
    import concourse.mybir as mybir
    import concourse.tile as tile
    from concourse import bacc

    f32 = mybir.dt.float32
    f32r = mybir.dt.float32r
    AF = mybir.ActivationFunctionType
    AF_SILU = getattr(AF, _ACT_NAME)
    ET = mybir.EngineType

    nc = bacc.Bacc("TRN2", target_bir_lowering=False, debug=False)

    tvT = nc.dram_tensor("tvT", [D, TBLK], f32r, kind="ExternalInput")
    svT = nc.dram_tensor("svT", [D, S], f32r, kind="ExternalInput")
    wtT = nc.dram_tensor("wtT", [D, R], f32r, kind="ExternalInput")
    wsT2 = nc.dram_tensor("wsT2", [D, 128], f32r, kind="ExternalInput")
    wtb_col = nc.dram_tensor("wtb_col", [R + 1, 1], f32r, kind="ExternalInput")
    ws_rep256 = nc.dram_tensor("ws_rep256", [R, TBLK], f32r,
                               kind="ExternalInput")
    big = nc.dram_tensor("big", [128, 192], f32r, kind="ExternalInput")
    bias_row = nc.dram_tensor("bias_row", [1, TBLK], f32r,
                              kind="ExternalInput")
    ones_row = nc.dram_tensor("ones_row", [1, S], f32r, kind="ExternalInput")
    out = nc.dram_tensor("out", [TBLK, S], f32, kind="ExternalOutput")

    with tile.TileContext(nc) as tc:
        with (
            tc.tile_pool(name="const", bufs=1) as cpool,
            tc.tile_pool(name="ptb", bufs=2) as ptbpool,
            tc.tile_pool(name="prod", bufs=2) as prodpool,
            tc.tile_pool(name="actb",
                         bufs=(4 if variant == "act_fused" else 2)) as actpool,
            tc.tile_pool(name="ps2_psum", bufs=1, space="PSUM") as ps2pool,
            tc.tile_pool(name="pt_psum", bufs=1, space="PSUM") as ptpool,
            tc.tile_pool(name="tl_psum", bufs=1, space="PSUM") as tlpool,
            tc.tile_pool(name="score_psum", bufs=2, space="PSUM") as spool,
            tc.tile_pool(name="outsb", bufs=2) as outpool,
        ):
            def emit_body():
                wtT_sb = cpool.tile([128, 4 * R], f32r, tag="wtT_sb")
                wsT2_sb = cpool.tile([128, 4 * 128], f32r, tag="wsT2_sb")
                wtb_sb = cpool.tile([R + 1, 1], f32r, tag="wtb_sb")
                slt_stat = cpool.tile([R + 1, TBLK], f32r, tag="slt_stat")
                big_sb = cpool.tile([128, 192], f32r, tag="big_sb")
                tv_sb = cpool.tile([128, 4 * TBLK], f32r, tag="tv_sb")
                sv_k = [cpool.tile([128, S], f32r, tag=f"sv_{k}",
                                   name=f"sv_{k}")
                        for k in range(4)]
                ps2_sb = cpool.tile([128, S], f32, tag="ps2_sb")
                psl = cpool.tile([R + 1, S], f32r, tag="psl")
                pt_sb = cpool.tile([R + 1, TBLK], f32r, tag="pt_sb")

                for k in range(4):
                    nc.sync.dma_start(out=sv_k[k][:],
                                      in_=svT[k * 128:(k + 1) * 128, :])
                    nc.sync.dma_start(out=wtT_sb[:, k * R:(k + 1) * R],
                                      in_=wtT[k * 128:(k + 1) * 128, :])
                    nc.sync.dma_start(out=wsT2_sb[:, k * 128:(k + 1) * 128],
                                      in_=wsT2[k * 128:(k + 1) * 128, :])
                    nc.sync.dma_start(out=tv_sb[:, k * TBLK:(k + 1) * TBLK],
                                      in_=tvT[k * 128:(k + 1) * 128, :])
                nc.sync.dma_start(out=wtb_sb[:], in_=wtb_col[:])
                nc.sync.dma_start(out=slt_stat[0:R, :], in_=ws_rep256[:])
                nc.sync.dma_start(out=big_sb[:], in_=big[:])
                nc.sync.dma_start(out=pt_sb[R:R + 1, :], in_=bias_row[:])

                # ---- projections on PE (float32r, full rate) ----
                # ps2 directly in PSUM, r duplicated on partition halves via
                # the host-duplicated stationary wsT2.
                ps2 = ps2pool.tile([128, S], f32, tag="ps2")
                for kc in range(4):
                    for nh in range(2):
                        nc.tensor.matmul(
                            ps2[:, nh * 512:(nh + 1) * 512],
                            (wsT2_sb[:, kc * 128:(kc + 1) * 128]),
                            (sv_k[kc][:, nh * 512:(nh + 1) * 512]),
                            start=(kc == 0), stop=(kc == 3))
                pt_ps = ptpool.tile([R, TBLK], f32, tag="pt_ps")
                for kc in range(4):
                    nc.tensor.matmul(
                        pt_ps[:],
                        (wtT_sb[:, kc * R:(kc + 1) * R]),
                        (tv_sb[:, kc * TBLK:(kc + 1) * TBLK]),
                        start=(kc == 0), stop=(kc == 3))
                # SBUF copies: full duplicated ps2 (ACT input), psl (rows
                # 0:64 = ps + a ones row 64) for the psum-init matmul, pt
                nc.vector.tensor_copy(ps2_sb[:], ps2[:])
                nc.vector.tensor_copy(psl[0:R, :], ps2[0:R, :])
                nc.sync.dma_start(out=psl[R:R + 1, :], in_=ones_row[:])
                nc.vector.tensor_copy(pt_sb[0:R, :], pt_ps[:])

                # tl+bias row: one matmul over [65,(pt;bias_row)] -> [1, 256]
                tl_ps = tlpool.tile([1, TBLK], f32, tag="tl_ps")
                nc.tensor.matmul(tl_ps[:], (wtb_sb[:]), (pt_sb[:]),
                                 start=True, stop=True)
                nc.vector.tensor_copy(slt_stat[R:R + 1, :], tl_ps[:])

                for tb in range(2):
                    ptb2 = ptbpool.tile([128, R], f32, tag="ptb2")
                    nc.vector.tensor_copy(ptb2[0:R, :],
                                          pt_sb[0:R, tb * 128: tb * 128 + R])
                    nc.vector.tensor_copy(
                        ptb2[R:128, :],
                        pt_sb[0:R, tb * 128 + R: tb * 128 + 128])

                    score_ps = spool.tile([128, S], f32, tag="score_ps")
                    # init psum with sl[s] + tl[t] + bias in one matmul
                    for nh in range(2):
                        nc.tensor.matmul(
                            score_ps[:, nh * 512:(nh + 1) * 512],
                            (slt_stat[:, tb * 128:(tb + 1) * 128]),
                            (psl[:, nh * 512: nh * 512 + 512]),
                            start=True, stop=False)

                    if variant == "act_fused":
                        for p in range(64):
                            actb = actpool.tile([128, S], f32r, tag="actb")
                            nc.scalar.activation(actb[:], ps2_sb[:], AF_SILU,
                                                 scale=ptb2[:, p:p + 1])
                            last = (p == 63)
                            for nh in range(2):
                                nc.tensor.matmul(
                                    score_ps[:, nh * 512:(nh + 1) * 512],
                                    (big_sb[:, 63 - p: 63 - p + 128]),
                                    (actb[:, nh * 512: nh * 512 + 512]),
                                    start=False, stop=last)
                    else:  # dve_prod
                        npair = 64 // GRP
                        for g in range(GRP):
                            prod = prodpool.tile([128, npair * S], f32,
                                                 tag="prod")
                            for j in range(npair):
                                p = g * npair + j
                                nc.vector.tensor_scalar_mul(
                                    prod[:, j * S:(j + 1) * S],
                                    ps2_sb[:],
                                    ptb2[:, p:p + 1])
                            actb = actpool.tile([128, npair * S], f32r,
                                                tag="actb")
                            nc.scalar.activation(actb[:], prod[:], AF_SILU)
                            for j in range(npair):
                                p = g * npair + j
                                last = (g == GRP - 1 and j == npair - 1)
                                for nh in range(2):
                                    nc.tensor.matmul(
                                        score_ps[:, nh * 512:(nh + 1) * 512],
                                        (big_sb[:, 63 - p: 63 - p + 128]),
                                        (actb[:, j * S + nh * 512:
                                                j * S + nh * 512 + 512]),
                                        start=False, stop=last)

                    out_sb = outpool.tile([128, S], f32, tag="out_sb")
                    nc.vector.tensor_copy(out_sb[:], score_ps[:])
                    nc.sync.dma_start(out=out[tb * 128:(tb + 1) * 128, :],
                                      in_=out_sb[:])

            if loop_n > 0:
                with tc.For_i(0, loop_n, 1,
                              hint_engines=(ET.Activation, ET.PE)):
                    emit_body()
            else:
                emit_body()
    nc.compile()
    return nc


def _get_nc(variant=VARIANT, loop_n=0):
    key = (variant, loop_n, _ACT_NAME)
    if key not in _compiled:
        _compiled[key] = _build_nc(variant=variant, loop_n=loop_n)
    return _compiled[key]


def make_in_maps(target_val, source_val, Wt, Ws, wt_out, ws_out, iw, bias_f):
    wtT = np.ascontiguousarray(Wt.T)                      # [D, R]
    wsT = np.ascontiguousarray(Ws.T)                      # [D, R]
    wsT2 = np.ascontiguousarray(np.concatenate([wsT, wsT], axis=1))  # [D,128]
    wtb_col = np.ascontiguousarray(
        np.concatenate([wt_out, np.ones(1, np.float32)])[:, None])  # [R+1,1]
    ws_rep256 = np.ascontiguousarray(
        np.broadcast_to(ws_out[:, None], (R, TBLK)))      # [R, 256]
    big = np.zeros((128, 192), dtype=np.float32)
    big[0:R, 63] = iw
    big[R:128, 127] = iw
    bias_row = np.full((1, TBLK), bias_f, dtype=np.float32)

    svT = [np.ascontiguousarray(source_val[b].T) for b in range(B)]

    in_maps = []
    for c in range(NCORES):
        b, ti = c // 4, c % 4
        in_maps.append({
            "tvT": np.ascontiguousarray(
                target_val[b, ti * TBLK:(ti + 1) * TBLK, :].T),
            "svT": svT[b],
            "wtT": wtT,
            "wsT2": wsT2,
            "wtb_col": wtb_col,
            "ws_rep256": ws_rep256,
            "big": big,
            "bias_row": bias_row,
            "ones_row": np.ones((1, S), dtype=np.float32),
        })
    return in_maps


def kernel(target_val, source_val, Wt, Ws, wt_out, ws_out,
           interaction_weight, bias):
    from concourse.bass_utils import run_bass_kernel_spmd

    target_val = np.asarray(target_val, dtype=np.float32)
    source_val = np.asarray(source_val, dtype=np.float32)
    Wt = np.asarray(Wt, dtype=np.float32)
    Ws = np.asarray(Ws, dtype=np.float32)
    wt_out = np.asarray(wt_out, dtype=np.float32)
    ws_out = np.asarray(ws_out, dtype=np.float32)
    iw = np.asarray(interaction_weight, dtype=np.float32)
    bias_f = float(np.asarray(bias, dtype=np.float32))

    nc = _get_nc()
    in_maps = make_in_maps(target_val, source_val, Wt, Ws, wt_out, ws_out,
                           iw, bias_f)
    res = run_bass_kernel_spmd(nc, in_maps, core_ids=list(range(NCORES)))

    scores = np.empty((B, T, S), dtype=np.float32)
    for c in range(NCORES):
        b, ti = c // 4, c % 4
        scores[b, ti * TBLK:(ti + 1) * TBLK, :] = res.results[c]["out"]
    return scores

